# revision 3
# baseline (speedup 1.0000x reference)
"""Trainium2 Bass kernel for a ClassificationHead — v2.

Math (per token over e=768):
  g2   = gamma*W0 - mean-fold;  c = beta.W0 + bias
  s2   = dot(x, g2);  var = E[x^2] - E[x]^2
  out  = sigmoid(s2 / sqrt(var+eps) + c)

v2 changes over the 101.5us baseline:
  - Sqrt table warmed once during the DMA-wait window; the main stream
    (Square/Copy) loads no ACT tables, so it survives to the epilogue.
    Only one table load (Sigmoid) remains, on the tail.
  - bn_aggr (26 x 120ns DVE) replaced with a batched mean/var combine
    on the raw bn_stats fields ([c,m,c*var] x even/odd) for all bn
    columns in a handful of wide ops split across ACT/DVE.
  - First x tile loads as two 1-column DMAs so DVE starts ~1us earlier.
  - Half-0 epilogue and result DMA issue mid-stream; only half-1 sits
    on the tail.
"""

import os

import numpy as np

import concourse.bacc as bacc
import concourse.bass as bass
import concourse.tile as tile
from concourse import mybir
from concourse.bass_utils import run_bass_kernel_spmd

B, N, E = 256, 257, 768
N_CORES = 8
BS = B // N_CORES          # batches per core
T = BS * (N - 1)           # tokens per core = 8192
P = 128                    # partitions
S = T // P                 # stat columns per core = 64
EPS = 1e-5
SUBW = 192.0               # bn_stats sub-group width (384 split even/odd)

_CACHE = {}
LAST_RESULTS = None


def _build_nc():
    nc = bacc.Bacc(None, target_bir_lowering=False)
    f32 = mybir.dt.float32
    J = 2                       # columns per DMA
    G = 8                       # column group size for the bn/ACT pattern
    K = 3                       # bn columns per group
    NH = 2                      # epilogue halves
    SH = S // NH                # columns per half = 32
    NGH = SH // G               # groups per half = 4
    NBH = NGH * K               # bn columns per half = 12
    n_act = G - K

    x = nc.dram_tensor("x", [T, E], f32, kind="ExternalInput")
    params = nc.dram_tensor("params", [P, E + 1], f32, kind="ExternalInput")
    out = nc.dram_tensor("out", [T], f32, kind="ExternalOutput")
    x_rj = x.ap().rearrange("(p s j) e -> s p (j e)", p=P, j=J)
    out_r = out.ap().rearrange("(p s) -> p s", p=P)

    with tile.TileContext(nc) as tc:
        with (
            tc.tile_pool(name="singles", bufs=1) as singles,
            tc.tile_pool(name="loads", bufs=8) as loads,
            tc.tile_pool(name="work", bufs=3) as work,
            tc.tile_pool(name="stats", bufs=1) as stats_pool,
            tc.tile_pool(name="accums", bufs=1, space="PSUM") as accums,
        ):
            params_t = singles.tile([P, E + 1], f32)
            g2_t = params_t[:, 0:E]
            c_ap = params_t[:, E : E + 1]
            eps_t = singles.tile([P, 1], f32)
            nc.vector.memset(eps_t, EPS)

            # Warm ONLY the Sqrt table: Square/Copy in the main stream load
            # no tables, so Sqrt survives until the epilogue needs it.
            warm = singles.tile([P, 1], f32)
            nc.scalar.activation(
                out=warm, in_=eps_t,
                func=mybir.ActivationFunctionType.Sqrt, bias=eps_t, scale=1.0,
            )

            s2 = [stats_pool.tile([P, SH], f32, name=f"s2_{h}") for h in range(NH)]
            # raw bn_stats output, [P, col, 2 insts, 6 fields]
            st = [
                stats_pool.tile([P, NBH, 2, 6], f32, name=f"st_{h}")
                for h in range(NH)
            ]
            sm = [accums.tile([P, NGH, n_act], f32, name=f"sm_{h}") for h in range(NH)]
            sq = [accums.tile([P, NGH, n_act], f32, name=f"sq_{h}") for h in range(NH)]
            res_all = stats_pool.tile([P, S], f32, name="res_all")

            def epilogue(h):
                # Batched bn combine for the 12 bn columns of this half:
                # fields per 6-block: [c,m,c*var] for even/odd element sets,
                # counts are all 192. mean = avg of 4 means;
                # E[x^2] = avg of (c*var)/192 + m^2 over the 4 sub-groups.
                sth = st[h]
                # m and cv as [P, NBH, 2, 2] strided views (fields {1,4}, {2,5})
                m_ap = sth[:, :, :, 1:5:3]
                cv_ap = sth[:, :, :, 2:6:3]
                msq = stats_pool.tile([P, NBH, 2, 2], f32, name=f"msq_{h}")
                nc.scalar.activation(
                    out=msq, in_=m_ap, func=mybir.ActivationFunctionType.Square,
                )
                q = stats_pool.tile([P, NBH, 2, 2], f32, name=f"q_{h}")
                # q = cv/SUBW + m^2
                nc.vector.scalar_tensor_tensor(
                    out=q, in0=cv_ap, scalar=1.0 / SUBW, in1=msq,
                    op0=mybir.AluOpType.mult, op1=mybir.AluOpType.add,
                )
                mu_bn = stats_pool.tile([P, NBH], f32, name=f"mu_bn_{h}")
                nc.vector.tensor_reduce(
                    out=mu_bn, in_=m_ap, axis=mybir.AxisListType.XY,
                    op=mybir.AluOpType.add,
                )
                q_bn = stats_pool.tile([P, NBH], f32, name=f"q_bn_{h}")
                nc.vector.tensor_reduce(
                    out=q_bn, in_=q, axis=mybir.AxisListType.XY,
                    op=mybir.AluOpType.add,
                )
                # var_bn = q_bn/4 - (mu_bn/4)^2 ; do via musq then STT
                mubn_sq = stats_pool.tile([P, NBH], f32, name=f"mubnsq_{h}")
                nc.scalar.activation(
                    out=mubn_sq, in_=mu_bn,
                    func=mybir.ActivationFunctionType.Square, scale=0.25,
                )
                var = stats_pool.tile([P, NGH, G], f32, name=f"var_{h}")
                nc.vector.scalar_tensor_tensor(
                    out=var[:, :, 0:K],
                    in0=q_bn.rearrange("p (a b) -> p a b", a=NGH),
                    scalar=0.25,
                    in1=mubn_sq.rearrange("p (a b) -> p a b", a=NGH),
                    op0=mybir.AluOpType.mult, op1=mybir.AluOpType.subtract,
                )

                # ACT columns: var = sq/E - (sm/E)^2
                mu = stats_pool.tile([P, NGH, n_act], f32, name=f"mu_{h}")
                nc.scalar.activation(
                    out=mu, in_=sm[h],
                    func=mybir.ActivationFunctionType.Copy, scale=1.0 / E,
                )
                musq = stats_pool.tile([P, NGH, n_act], f32, name=f"musq_{h}")
                nc.scalar.activation(
                    out=musq, in_=mu, func=mybir.ActivationFunctionType.Square,
                )
                nc.vector.scalar_tensor_tensor(
                    out=var[:, :, K:G], in0=sq[h], scalar=1.0 / E, in1=musq,
                    op0=mybir.AluOpType.mult, op1=mybir.AluOpType.subtract,
                )
                varf = var.rearrange("p a b -> p (a b)")
                std = stats_pool.tile([P, SH], f32, name=f"std_{h}")
                nc.scalar.activation(
                    out=std, in_=varf,
                    func=mybir.ActivationFunctionType.Sqrt, bias=eps_t, scale=1.0,
                )
                rstd = stats_pool.tile([P, SH], f32, name=f"rstd_{h}")
                nc.vector.reciprocal(out=rstd, in_=std)
                logit = stats_pool.tile([P, SH], f32, name=f"logit_{h}")
                nc.vector.tensor_mul(out=logit, in0=s2[h], in1=rstd)
                nc.scalar.activation(
                    out=res_all[:, h * SH : (h + 1) * SH], in_=logit,
                    func=mybir.ActivationFunctionType.Sigmoid,
                    bias=c_ap, scale=1.0,
                )
                nc.sync.dma_start(
                    out=out_r[:, h * SH : (h + 1) * SH],
                    in_=res_all[:, h * SH : (h + 1) * SH],
                )

            for s in range(S // J):
                x_t = loads.tile([P, J * E], f32)
                if s == 0:
                    # two 1-column DMAs: col 0 lands sooner, DVE starts earlier
                    for j in range(J):
                        nc.sync.dma_start(
                            out=x_t[:, j * E : (j + 1) * E],
                            in_=x_rj[s][:, j * E : (j + 1) * E],
                        )
                    nc.sync.dma_start(out=params_t, in_=params.ap())
                else:
                    nc.sync.dma_start(out=x_t, in_=x_rj[s])

                for j in range(J):
                    col = J * s + j
                    h, ch = col // SH, col % SH
                    g, i = ch // G, ch % G
                    xj = x_t[:, j * E : (j + 1) * E]

                    if i < K:
                        bcol = g * K + i
                        x2 = xj.rearrange("p (w f) -> p w f", w=2)
                        for w in range(2):
                            nc.vector.bn_stats(
                                out=st[h][:, bcol, w, :], in_=x2[:, w, :]
                            )
                    else:
                        ac = i - K
                        d_sq = work.tile([P, 1], f32, tag="d_sq")
                        nc.scalar.activation(
                            out=d_sq.broadcast_to(xj.shape), in_=xj,
                            func=mybir.ActivationFunctionType.Square,
                            accum_out=sq[h][:, g, ac : ac + 1],
                        )
                        d_sm = work.tile([P, 1], f32, tag="d_sm")
                        nc.scalar.activation(
                            out=d_sm.broadcast_to(xj.shape), in_=xj,
                            func=mybir.ActivationFunctionType.Copy,
                            accum_out=sm[h][:, g, ac : ac + 1],
                        )

                    d = work.tile([P, 1], f32, tag="d")
                    nc.vector.scalar_tensor_tensor(
                        out=d.broadcast_to(xj.shape), in0=xj, scalar=1.0,
                        in1=g2_t,
                        op0=mybir.AluOpType.mult, op1=mybir.AluOpType.mult,
                        accum_out=s2[h][:, ch : ch + 1],
                    )
                if s == (S // J) // 2 - 1:
                    epilogue(0)

            epilogue(1)

    nc.compile()
    return nc


def kernel(x, ln_gamma, ln_beta, W, bias):
    global LAST_RESULTS
    x = np.ascontiguousarray(np.asarray(x, dtype=np.float32))
    ln_gamma = np.asarray(ln_gamma, dtype=np.float32)
    ln_beta = np.asarray(ln_beta, dtype=np.float32)
    W = np.asarray(W, dtype=np.float32)
    bias = np.asarray(bias, dtype=np.float32)

    geff = ln_gamma * W[0]
    g2 = geff - geff.sum() / E
    c = float(ln_beta @ W[0] + bias[0])

    params = np.empty((P, E + 1), dtype=np.float32)
    params[:, :E] = g2[None, :]
    params[:, E] = c

    h = x[:, 1:, :]
    shards = [
        np.ascontiguousarray(h[i * BS : (i + 1) * BS].reshape(T, E))
        for i in range(N_CORES)
    ]

    if "nc" not in _CACHE:
        _CACHE["nc"] = _build_nc()
    nc = _CACHE["nc"]

    in_maps = [{"x": shards[i], "params": params} for i in range(N_CORES)]
    trace = bool(int(os.environ.get("BASS_KERNEL_TRACE", "0")))
    results = run_bass_kernel_spmd(
        nc, in_maps, core_ids=list(range(N_CORES)), trace=trace
    )
    LAST_RESULTS = results

    outs = [results.results[i]["out"] for i in range(N_CORES)]
    full = np.concatenate(outs).reshape(B, N - 1, 1).astype(np.float32)
    return full
